# revision 20
# baseline (speedup 1.0000x reference)
"""CAWN resonance block on 8 TRN2 NeuronCores — data-parallel over batch.

Per core (one batch sample, S=4096, D=1024, R=128):
  Phase A (16 chunks of 256 seq positions):
    rmsnorm1 (Act square+accum, rsqrt via DVE Newton) -> xhat
    -> TensorE transpose to (d,s) -> depthwise conv as diag-matmul PSUM
    accumulation (fp32r) -> silu per k-block (conv bias folded into Act
    bias, accum_out -> x_mean) -> L2 row norms via all-ones matmul;
    1/||h||^2 rows staged for phase B.
  Phase B: gate/retention thin matmuls + sigmoid; one Act Sqrt turns staged
    1/||h||^2 into 1/||h|| rows, re-broadcast via rank-1 fp32r matmuls;
    sig = resT*gate*rl2 in one fused stt; tensor_tensor_scan recurrence.
  Phase C (16 pairs of 128-row tiles):
    out-proj (fp32r) -> residual -> rmsnorm2 (Newton rsqrt; rstd folded
    into silu scale, beta@W.T seeded into PSUM via rank-1 matmul)
    -> FFN (fp32r) -> silu -> residual.

Activation-table discipline: only Square/Copy/Silu (one table set) inside
the loops; Sqrt and Sigmoid appear once each in phase B.
"""

import os
import numpy as np
from contextlib import ExitStack

B, S, D, R = 8, 4096, 1024, 128
NK = D // 128            # 8 partition-blocks of D
CH = 256                 # phase A chunk (seq)
NCH = S // CH            # 16
ST = 128                 # phase C tile (seq)
NST = S // ST            # 32
SC = 512                 # scan chunk
NSC = S // SC            # 8
EPS = float(np.finfo(np.float32).eps)

RES_FR = os.environ.get('RES_FR', '0') == '1'           # fp32r for the resonance matmul (feeds new_state)

_CACHE = {}


def _build():
    import concourse.bacc as bacc
    import concourse.tile as tile
    from concourse import masks, mybir

    F = mybir.dt.float32
    FR = mybir.dt.float32r
    BF = mybir.dt.bfloat16
    AF = mybir.ActivationFunctionType
    OP = mybir.AluOpType

    nc = bacc.Bacc("TRN2", target_bir_lowering=False, debug=False)

    x_d = nc.dram_tensor("x", (S, D), F, kind="ExternalInput")
    st0_d = nc.dram_tensor("state0", (R, 1), F, kind="ExternalInput")
    taps_d = nc.dram_tensor("taps", (128, NK, 3), F, kind="ExternalInput")
    cbias_d = nc.dram_tensor("cbias", (128, NK), F, kind="ExternalInput")
    corr_d = nc.dram_tensor("corr", (128, NK, 2), F, kind="ExternalInput")
    fnT_d = nc.dram_tensor("fnT", (D, R), F, kind="ExternalInput")
    wgT_d = nc.dram_tensor("wgT", (D, R), F, kind="ExternalInput")
    wrT_d = nc.dram_tensor("wrT", (D, R), F, kind="ExternalInput")
    gbias_d = nc.dram_tensor("gbias", (R, 1), F, kind="ExternalInput")
    rbias_d = nc.dram_tensor("rbias", (R, 1), F, kind="ExternalInput")
    woT_d = nc.dram_tensor("woT", (R, D), F, kind="ExternalInput")
    wfT_d = nc.dram_tensor("wfT", (D, D), F, kind="ExternalInput")
    fbias_d = nc.dram_tensor("fbias", (1, D), F, kind="ExternalInput")

    y_d = nc.dram_tensor("y", (S, D), F, kind="ExternalOutput")
    stN_d = nc.dram_tensor("stateN", (R, 1), F, kind="ExternalOutput")

    RD = FR if RES_FR else F

    with tile.TileContext(nc) as tc, ExitStack() as ctx:
        pers = ctx.enter_context(tc.tile_pool(name="pers", bufs=1))
        ident = pers.tile([128, 128], F)
        masks.make_identity(nc, ident[:])
        ident_r = pers.tile([128, 128], FR)
        nc.vector.tensor_copy(ident_r[:], ident[:])
        accTr = pers.tile([R, S], FR)       # fp32r copy for out-proj lhsT

        abctx = ExitStack()
        pab = abctx.enter_context(tc.tile_pool(name="pab", bufs=1))
        taps_t = pab.tile([128, NK, 3], F)
        nc.sync.dma_start(taps_t[:], taps_d.ap())
        cbias_t = pab.tile([128, NK], F)
        nc.sync.dma_start(cbias_t[:], cbias_d.ap())
        corr_t = pab.tile([128, NK, 2], F)
        nc.sync.dma_start(corr_t[:], corr_d.ap())
        fnT_t = pab.tile([128, NK, R], RD)
        if RES_FR:
            fnT_f = pab.tile([128, NK, R], F)
        wgT_t = pab.tile([128, NK, R], F)
        wrT_t = pab.tile([128, NK, R], F)
        for k in range(NK):
            if RES_FR:
                nc.sync.dma_start(fnT_f[:, k, :], fnT_d.ap()[k * 128:(k + 1) * 128, :])
                nc.vector.tensor_copy(fnT_t[:, k, :], fnT_f[:, k, :])
            else:
                nc.sync.dma_start(fnT_t[:, k, :], fnT_d.ap()[k * 128:(k + 1) * 128, :])
            nc.sync.dma_start(wgT_t[:, k, :], wgT_d.ap()[k * 128:(k + 1) * 128, :])
            nc.sync.dma_start(wrT_t[:, k, :], wrT_d.ap()[k * 128:(k + 1) * 128, :])
        gbias_t = pab.tile([R, 1], F)
        nc.sync.dma_start(gbias_t[:], gbias_d.ap())
        rbias_t = pab.tile([R, 1], F)
        nc.sync.dma_start(rbias_t[:], rbias_d.ap())
        st0_t = pab.tile([R, 1], F)
        nc.sync.dma_start(st0_t[:], st0_d.ap())

        diag_t = pab.tile([128, 3, NK, 128], FR)
        for k in range(3):
            for kb in range(NK):
                nc.vector.tensor_scalar(
                    diag_t[:, k, kb, :], ident[:], taps_t[:, kb, k:k + 1], None,
                    OP.mult)

        zb_t = pab.tile([128, NK, 2], F)
        nc.gpsimd.memset(zb_t[:], 0.0)
        onesb_t = pab.tile([128, 128], BF)
        onef = pab.tile([128, 128], F)
        nc.vector.memset(onef[:], 1.0)
        nc.vector.tensor_copy(onesb_t[:], onef[:])
        onecol_t = pab.tile([1, 128], FR)
        nc.vector.tensor_copy(onecol_t[:], onef[0:1, :])

        resT = pab.tile([R, S], F)         # resonance.T (unscaled)
        accT = pab.tile([R, S], F)         # scan output
        xmean = pab.tile([128, NK], F)
        nc.vector.memset(xmean[:], 0.0)
        p2row = pab.tile([1, S], F)        # 1/||h||^2 rows (staged)

        def newton_rsqrt(pool, ssq_ap, n, tag):
            """rsqrt(ssq/D + eps) via Newton from y0=1.5-0.5m (m near 1)."""
            m = pool.tile([128, n], F, tag=tag + "m")
            nc.vector.tensor_scalar(m[:], ssq_ap, 1.0 / D, EPS, OP.mult, OP.add)
            y = pool.tile([128, n], F, tag=tag + "y")
            nc.vector.tensor_scalar(y[:], m[:], -0.5, 1.5, OP.mult, OP.add)
            t1 = pool.tile([128, n], F, tag=tag + "t1")
            t2 = pool.tile([128, n], F, tag=tag + "t2")
            for _ in range(2):
                nc.vector.tensor_tensor(t1[:], y[:], y[:], OP.mult)
                nc.vector.tensor_tensor(t2[:], t1[:], m[:], OP.mult)
                nc.vector.tensor_scalar(t1[:], t2[:], -0.5, 1.5, OP.mult, OP.add)
                nc.vector.tensor_tensor(y[:], y[:], t1[:], OP.mult)
            return m, y

        # ================= PHASE A =================
        with ExitStack() as actx:
            pa = actx.enter_context(tc.tile_pool(name="pa", bufs=2))
            pa_sq = actx.enter_context(tc.tile_pool(name="pa_sq", bufs=2))
            pa_sc = actx.enter_context(tc.tile_pool(name="pa_sc", bufs=1))
            pp_tr = actx.enter_context(
                tc.tile_pool(name="pp_tr", bufs=2, space="PSUM"))
            pp_conv = actx.enter_context(
                tc.tile_pool(name="pp_conv", bufs=2, space="PSUM"))
            pp_res = actx.enter_context(
                tc.tile_pool(name="pp_res", bufs=1, space="PSUM"))
            pp_l2 = actx.enter_context(
                tc.tile_pool(name="pp_l2", bufs=1, space="PSUM"))

            prev_xhT = None
            for c in range(NCH):
                xhT = pa.tile([128, NK, CH + 2], FR, tag="xhT")
                if prev_xhT is None:
                    nc.vector.tensor_copy(xhT[:, :, 0:2], zb_t[:])
                else:
                    nc.vector.tensor_copy(xhT[:, :, 0:2],
                                          prev_xhT[:, :, CH:CH + 2])
                xts = []
                ssqt = pa.tile([128, 2], F, tag="ssqt")
                for half in range(2):
                    s0 = c * CH + half * 128
                    xt = pa.tile([128, D], F, tag=f"xt{half}")
                    nc.sync.dma_start(xt[:], x_d.ap()[s0:s0 + 128, :])
                    sq_scr = pa_sc.tile([128, D], BF, tag="sqscr")
                    nc.scalar.activation(sq_scr[:], xt[:], AF.Square,
                                         accum_out=ssqt[:, half:half + 1])
                    xts.append(xt)
                _, rstdt = newton_rsqrt(pa, ssqt[:], 2, "nA")
                for half in range(2):
                    xh = pa.tile([128, D], FR, tag="xh")
                    nc.vector.tensor_scalar(xh[:], xts[half][:],
                                            rstdt[:, half:half + 1], None,
                                            OP.mult)
                    for g in range(2):
                        trp = pp_tr.tile([128, 4, 128], FR, tag="trp")
                        for j in range(4):
                            kb = g * 4 + j
                            nc.tensor.transpose(
                                trp[:, j, :], xh[:, kb * 128:(kb + 1) * 128],
                                ident_r[:])
                        eng = nc.vector if g == 0 else nc.scalar
                        if g == 0:
                            nc.vector.tensor_copy(
                                xhT[:, g * 4:(g + 1) * 4,
                                    2 + half * 128:2 + half * 128 + 128],
                                trp[:])
                        else:
                            nc.scalar.copy(
                                xhT[:, g * 4:(g + 1) * 4,
                                    2 + half * 128:2 + half * 128 + 128],
                                trp[:])
                hT = pa.tile([128, NK, CH], F, tag="hT")
                xm_st = pa.tile([128, NK], F, tag="xmst")
                for grp in range(2):
                    cps = pp_conv.tile([128, 4, CH], F, tag="cps")
                    for j in range(4):
                        kb = grp * 4 + j
                        for k in range(3):
                            nc.tensor.matmul(
                                cps[:, j, :], diag_t[:, k, kb, :],
                                xhT[:, kb, k:k + CH],
                                start=(k == 0), stop=(k == 2))
                    if c == 0:
                        nc.vector.tensor_tensor(
                            cps[:, :, 0:2], cps[:, :, 0:2],
                            corr_t[:, grp * 4:(grp + 1) * 4, :], OP.add)
                    for j in range(4):
                        kb = grp * 4 + j
                        nc.scalar.activation(
                            hT[:, kb, :], cps[:, j, :], AF.Silu,
                            bias=cbias_t[:, kb:kb + 1],
                            accum_out=xm_st[:, kb:kb + 1])
                nc.vector.tensor_tensor(xmean[:], xmean[:], xm_st[:], OP.add)
                sq2 = pa_sq.tile([128, NK, CH], BF, tag="sq2")
                nc.vector.tensor_tensor(sq2[:], hT[:], hT[:], OP.mult)
                l2p = pp_l2.tile([128, CH], F, tag="l2p")
                for kb in range(NK):
                    nc.tensor.matmul(l2p[:], onesb_t[:], sq2[:, kb, :],
                                     start=(kb == 0), stop=(kb == NK - 1))
                nc.vector.reciprocal(p2row[0:1, c * CH:(c + 1) * CH],
                                     l2p[0:1, :])
                rps = pp_res.tile([R, CH], F, tag="rps")
                if RES_FR:
                    hTr = pa_sq.tile([128, NK, CH], FR, tag="hTr")
                    nc.vector.tensor_copy(hTr[:], hT[:])
                    for kb in range(NK):
                        nc.tensor.matmul(rps[:], fnT_t[:, kb, :],
                                         hTr[:, kb, :],
                                         start=(kb == 0), stop=(kb == NK - 1))
                else:
                    for kb in range(NK):
                        nc.tensor.matmul(rps[:], fnT_t[:, kb, :],
                                         hT[:, kb, :],
                                         start=(kb == 0), stop=(kb == NK - 1))
                nc.scalar.copy(resT[:, c * CH:(c + 1) * CH], rps[:])
                prev_xhT = xhT

        _PH = os.environ.get("PHASES", "abc")
        # ================= PHASE B =================
        with ExitStack() as bctx:
          if "b" in _PH:
            pb = bctx.enter_context(tc.tile_pool(name="pb", bufs=2))
            pp_b = bctx.enter_context(
                tc.tile_pool(name="pp_b", bufs=2, space="PSUM"))
            pp_rl = bctx.enter_context(
                tc.tile_pool(name="pp_rl", bufs=2, space="PSUM"))
            rl2_fr = pb.tile([1, S], FR)
            nc.scalar.activation(rl2_fr[:], p2row[:], AF.Sqrt)
            xm_s = pb.tile([128, NK], F)
            nc.vector.tensor_scalar(xm_s[:], xmean[:], 1.0 / S, None, OP.mult)
            gps = pp_b.tile([R, 1], F)
            rps2 = pp_b.tile([R, 1], F)
            for k in range(NK):
                nc.tensor.matmul(gps[:], wgT_t[:, k, :], xm_s[:, k:k + 1],
                                 start=(k == 0), stop=(k == NK - 1))
            for k in range(NK):
                nc.tensor.matmul(rps2[:], wrT_t[:, k, :], xm_s[:, k:k + 1],
                                 start=(k == 0), stop=(k == NK - 1))
            gate_s = pb.tile([R, 1], F)
            nc.scalar.activation(gate_s[:], gps[:], AF.Sigmoid,
                                 bias=gbias_t[:, 0:1])
            gmask = pb.tile([R, 1], F)
            nc.vector.tensor_scalar(gmask[:], gate_s[:], 0.001, None, OP.is_ge)
            gate_t = pb.tile([R, 1], F)
            nc.vector.tensor_tensor(gate_t[:], gate_s[:], gmask[:], OP.mult)
            ret_t = pb.tile([R, 1], F)
            nc.scalar.activation(ret_t[:], rps2[:], AF.Sigmoid,
                                 bias=rbias_t[:, 0:1])
            ones_sc = pb.tile([R, SC], F)
            nc.vector.memset(ones_sc[:], 1.0)
            ret_b = pb.tile([R, SC], F)
            nc.vector.tensor_scalar(ret_b[:], ones_sc[:], ret_t[:, 0:1], None,
                                    OP.mult)
            for c in range(NSC):
                rl2b = pp_rl.tile([128, SC], F, tag="rl2b")
                nc.tensor.matmul(rl2b[:], onecol_t[:],
                                 rl2_fr[:, c * SC:(c + 1) * SC],
                                 start=True, stop=True)
                sig = pb.tile([R, SC], F, tag="sig")
                nc.vector.scalar_tensor_tensor(
                    sig[:], resT[:, c * SC:(c + 1) * SC], gate_t[:, 0:1],
                    rl2b[:], OP.mult, OP.mult)
                init_ap = st0_t[:, 0:1] if c == 0 else accT[:, c * SC - 1:c * SC]
                nc.vector.tensor_tensor_scan(
                    accT[:, c * SC:(c + 1) * SC], ret_b[:], sig[:], init_ap,
                    OP.mult, OP.add)
                nc.scalar.copy(accTr[:, c * SC:(c + 1) * SC],
                               accT[:, c * SC:(c + 1) * SC])
            nc.sync.dma_start(stN_d.ap(), accT[:, S - 1:S])

        abctx.close()

        # ================= PHASE C =================
        with ExitStack() as cctx:
          if "c" in _PH:
            pc = cctx.enter_context(tc.tile_pool(name="pc", bufs=2))
            pc_sq = cctx.enter_context(tc.tile_pool(name="pc_sq", bufs=2))
            pp_op = cctx.enter_context(
                tc.tile_pool(name="pp_op", bufs=1, space="PSUM"))
            pp_tr2 = cctx.enter_context(
                tc.tile_pool(name="pp_tr2", bufs=2, space="PSUM"))
            pp_ffn = cctx.enter_context(
                tc.tile_pool(name="pp_ffn", bufs=2, space="PSUM"))
            pcw = cctx.enter_context(tc.tile_pool(name="pcw", bufs=1))

            wfT_t = pcw.tile([128, NK, D], FR)
            for k in range(NK):
                wf_f = pc.tile([128, D], F, tag="wfld")
                nc.sync.dma_start(wf_f[:], wfT_d.ap()[k * 128:(k + 1) * 128, :])
                nc.scalar.copy(wfT_t[:, k, :], wf_f[:])
            fbias_t = pcw.tile([1, D], FR)
            fb_f = pc.tile([1, D], F, tag="fbld")
            nc.sync.dma_start(fb_f[:], fbias_d.ap())
            nc.vector.tensor_copy(fbias_t[:], fb_f[:])
            woT_t = pcw.tile([R, D], FR)
            wo_f = pc.tile([R, D], F, tag="wold")
            nc.sync.dma_start(wo_f[:], woT_d.ap())
            nc.vector.tensor_copy(woT_t[:], wo_f[:])

            for tp in range(NST // 2):
                x2s = []
                ssqp = pc.tile([128, 2], F, tag="ssqp")
                for half in range(2):
                    s0 = (tp * 2 + half) * ST
                    ops = pp_op.tile([128, D], F, tag="ops")
                    for n in range(2):
                        nc.tensor.matmul(ops[:, n * 512:(n + 1) * 512],
                                         accTr[:, s0:s0 + 128],
                                         woT_t[:, n * 512:(n + 1) * 512],
                                         start=True, stop=True)
                    xt2 = pc.tile([128, D], F, tag=f"xt2{half}")
                    nc.sync.dma_start(xt2[:], x_d.ap()[s0:s0 + 128, :])
                    x2 = pc.tile([128, D], F, tag=f"x2{half}")
                    nc.vector.tensor_tensor(x2[:], xt2[:], ops[:], OP.add)
                    sq_scr = pc_sq.tile([128, D], BF, tag="sqscr2")
                    nc.scalar.activation(sq_scr[:], x2[:], AF.Square,
                                         accum_out=ssqp[:, half:half + 1])
                    x2s.append(x2)
                mp, rstdp = newton_rsqrt(pc, ssqp[:], 2, "nC")
                sstp = pc.tile([128, 2], F, tag="sstp")
                nc.vector.reciprocal(sstp[:], rstdp[:])
                srows = []
                for half in range(2):
                    rowp = pp_tr2.tile([1, 128], F, tag="trp2")
                    nc.tensor.transpose(rowp[:], sstp[:, half:half + 1],
                                        ident[:])
                    sr = pc.tile([1, 128], FR, tag=f"srow{half}")
                    nc.vector.tensor_copy(sr[:], rowp[:])
                    srows.append(sr)
                for half in range(2):
                    x2 = x2s[half]
                    x2T = pc.tile([128, NK, 128], FR, tag="x2T")
                    for g in range(2):
                        trp = pp_tr2.tile([128, 4, 128], F, tag="trp2")
                        for j in range(4):
                            kb = g * 4 + j
                            nc.tensor.transpose(
                                trp[:, j, :], x2[:, kb * 128:(kb + 1) * 128],
                                ident[:])
                        nc.scalar.copy(x2T[:, g * 4:(g + 1) * 4, :], trp[:])
                    fps = pp_ffn.tile([128, D], F, tag="fps")
                    for n in range(2):
                        for k in range(NK):
                            nc.tensor.matmul(fps[:, n * 512:(n + 1) * 512],
                                             x2T[:, k, :],
                                             wfT_t[:, k, n * 512:(n + 1) * 512],
                                             start=(k == 0), stop=False)
                        nc.tensor.matmul(fps[:, n * 512:(n + 1) * 512],
                                         srows[half][:],
                                         fbias_t[:, n * 512:(n + 1) * 512],
                                         start=False, stop=True)
                    sil = pc.tile([128, D], F, tag="sil")
                    nc.scalar.activation(sil[:], fps[:], AF.Silu,
                                         scale=rstdp[:, half:half + 1])
                    yt = pc.tile([128, D], F, tag="yt")
                    nc.vector.tensor_tensor(yt[:], x2[:], sil[:], OP.add)
                    s0 = (tp * 2 + half) * ST
                    nc.sync.dma_start(y_d.ap()[s0:s0 + 128, :], yt[:])

    nc.compile()
    return nc


def _prep(inputs):
    f32 = np.float32
    x = np.asarray(inputs["x"], f32)
    gamma = np.asarray(inputs["scale_gamma"], f32)
    beta = np.asarray(inputs["scale_beta"], f32)
    st0 = np.asarray(inputs["resonance_state"], f32)
    cw = np.asarray(inputs["conv_w"], f32)[:, 0, :]          # (D, 3)
    freq = np.asarray(inputs["frequencies"], f32)
    rbias = np.asarray(inputs["retention_bias"], f32)
    w_ret = np.asarray(inputs["w_ret"], f32)
    w_gate = np.asarray(inputs["w_gate"], f32)
    gbias = np.asarray(inputs["gate_bias"], f32)
    w_out = np.asarray(inputs["w_out"], f32)
    w_ffn = np.asarray(inputs["w_ffn"], f32)
    n1 = np.asarray(inputs["norm1_w"], f32)
    n2 = np.asarray(inputs["norm2_w"], f32)

    g1 = n1 * gamma
    taps = cw * g1[:, None]                                   # (D, 3)
    cbias = beta * cw.sum(1)                                  # (D,)
    corr0 = -beta * (cw[:, 0] + cw[:, 1])
    corr1 = -beta * cw[:, 0]

    def dlay(v):  # (D,) -> (128, NK)
        return np.ascontiguousarray(v.reshape(NK, 128).T)

    taps_a = np.ascontiguousarray(
        np.stack([dlay(taps[:, k]) for k in range(3)], axis=-1))  # (128,NK,3)
    corr_a = np.ascontiguousarray(
        np.stack([dlay(corr0), dlay(corr1)], axis=-1))            # (128,NK,2)

    fn = freq / np.maximum(np.linalg.norm(freq, axis=-1, keepdims=True), 1e-12)
    g2 = n2 * gamma
    wfT = np.ascontiguousarray(w_ffn.T * g2[:, None])
    fbias = (beta @ w_ffn.T).reshape(1, D)

    shared = dict(
        taps=taps_a, cbias=np.ascontiguousarray(dlay(cbias)), corr=corr_a,
        fnT=np.ascontiguousarray(fn.T), wgT=np.ascontiguousarray(w_gate.T),
        wrT=np.ascontiguousarray(w_ret.T),
        gbias=np.ascontiguousarray(gbias.reshape(R, 1)),
        rbias=np.ascontiguousarray(rbias.reshape(R, 1)),
        woT=np.ascontiguousarray(w_out.T), wfT=wfT,
        fbias=np.ascontiguousarray(fbias))
    in_maps = []
    for b in range(B):
        m = dict(shared)
        m["x"] = np.ascontiguousarray(x[b])
        m["state0"] = np.ascontiguousarray(st0[b].reshape(R, 1))
        in_maps.append(m)
    return in_maps


def kernel(**inputs):
    from concourse.bass_utils import run_bass_kernel_spmd
    if "nc" not in _CACHE:
        _CACHE["nc"] = _build()
    nc = _CACHE["nc"]
    in_maps = _prep(inputs)
    res = run_bass_kernel_spmd(nc, in_maps, core_ids=list(range(B)))
    y = np.stack([res.results[b]["y"] for b in range(B)])
    stN = np.stack([res.results[b]["stateN"][:, 0] for b in range(B)])
    return y, stN


# revision 47
# speedup vs baseline: 1.0232x; 1.0232x over previous
"""CAWN resonance block on 8 TRN2 NeuronCores — data-parallel over batch.

Per core (one batch sample, S=4096, D=1024, R=128):
  Phase A (16 chunks of 256 seq positions):
    rmsnorm1 (Act square+accum, rsqrt via DVE Newton) -> xhat
    -> TensorE transpose to (d,s) -> depthwise conv as diag-matmul PSUM
    accumulation (fp32r) -> silu per k-block (conv bias folded into Act
    bias, accum_out -> x_mean) -> L2 row norms via all-ones matmul;
    1/||h||^2 rows staged for phase B.
  Phase B: gate/retention thin matmuls + sigmoid; one Act Sqrt turns staged
    1/||h||^2 into 1/||h|| rows, re-broadcast via rank-1 fp32r matmuls;
    sig = resT*gate*rl2 in one fused stt; tensor_tensor_scan recurrence.
  Phase C (16 pairs of 128-row tiles):
    out-proj (fp32r) -> residual -> rmsnorm2 (Newton rsqrt; rstd folded
    into silu scale, beta@W.T seeded into PSUM via rank-1 matmul)
    -> FFN (fp32r) -> silu -> residual.

Activation-table discipline: only Square/Copy/Silu (one table set) inside
the loops; Sqrt and Sigmoid appear once each in phase B.
"""

import os
import numpy as np
from contextlib import ExitStack

B, S, D, R = 8, 4096, 1024, 128
NK = D // 128            # 8 partition-blocks of D
CH = 256                 # phase A chunk (seq)
NCH = S // CH            # 16
ST = 128                 # phase C tile (seq)
NST = S // ST            # 32
SC = 1024                # scan chunk
NSC = S // SC            # 8
EPS = float(np.finfo(np.float32).eps)

RES_FR = os.environ.get('RES_FR', '1') == '1'           # fp32r for the resonance matmul (feeds new_state)

_CACHE = {}


def _build():
    import concourse.bacc as bacc
    import concourse.tile as tile
    from concourse import masks, mybir

    F = mybir.dt.float32
    FR = mybir.dt.float32r
    BF = mybir.dt.bfloat16
    AF = mybir.ActivationFunctionType
    OP = mybir.AluOpType

    nc = bacc.Bacc("TRN2", target_bir_lowering=False, debug=False)

    x_d = nc.dram_tensor("x", (S, D), F, kind="ExternalInput")
    st0_d = nc.dram_tensor("state0", (R, 1), F, kind="ExternalInput")
    taps_d = nc.dram_tensor("taps", (128, NK, 3), F, kind="ExternalInput")
    cbias_d = nc.dram_tensor("cbias", (128, NK), F, kind="ExternalInput")
    corr_d = nc.dram_tensor("corr", (128, NK, 2), F, kind="ExternalInput")
    fnT_d = nc.dram_tensor("fnT", (D, R), F, kind="ExternalInput")
    wgT_d = nc.dram_tensor("wgT", (D, R), F, kind="ExternalInput")
    wrT_d = nc.dram_tensor("wrT", (D, R), F, kind="ExternalInput")
    gbias_d = nc.dram_tensor("gbias", (R, 1), F, kind="ExternalInput")
    rbias_d = nc.dram_tensor("rbias", (R, 1), F, kind="ExternalInput")
    woT_d = nc.dram_tensor("woT", (R, D), F, kind="ExternalInput")
    wfT_d = nc.dram_tensor("wfT", (D, D), F, kind="ExternalInput")
    fbias_d = nc.dram_tensor("fbias", (1, D), F, kind="ExternalInput")

    y_d = nc.dram_tensor("y", (S, D), F, kind="ExternalOutput")
    stN_d = nc.dram_tensor("stateN", (R, 1), F, kind="ExternalOutput")

    RD = FR if RES_FR else F

    with tile.TileContext(nc) as tc, ExitStack() as ctx:
        pers = ctx.enter_context(tc.tile_pool(name="pers", bufs=1))
        ident = pers.tile([128, 128], F)
        masks.make_identity(nc, ident[:])
        ident_r = pers.tile([128, 128], FR)
        nc.vector.tensor_copy(ident_r[:], ident[:])
        accTr = pers.tile([R, S], FR)       # fp32r copy for out-proj lhsT

        abctx = ExitStack()
        pab = abctx.enter_context(tc.tile_pool(name="pab", bufs=1))
        taps_t = pab.tile([128, NK, 3], F)
        nc.sync.dma_start(taps_t[:], taps_d.ap())
        cbias_t = pab.tile([128, NK], F)
        nc.sync.dma_start(cbias_t[:], cbias_d.ap())
        corr_t = pab.tile([128, NK, 2], F)
        nc.sync.dma_start(corr_t[:], corr_d.ap())
        fnT_t = pab.tile([128, NK, R], RD)
        if RES_FR:
            fnT_f = pab.tile([128, NK, R], F)
        wgT_t = pab.tile([128, NK, R], F)
        wrT_t = pab.tile([128, NK, R], F)
        for k in range(NK):
            if RES_FR:
                nc.sync.dma_start(fnT_f[:, k, :], fnT_d.ap()[k * 128:(k + 1) * 128, :])
                nc.vector.tensor_copy(fnT_t[:, k, :], fnT_f[:, k, :])
            else:
                nc.sync.dma_start(fnT_t[:, k, :], fnT_d.ap()[k * 128:(k + 1) * 128, :])
        gbias_t = pab.tile([R, 1], F)
        rbias_t = pab.tile([R, 1], F)
        st0_t = pab.tile([R, 1], F)

        diag_t = pab.tile([128, 3, NK, 128], FR)
        for k in range(3):
            for kb in range(NK):
                nc.vector.tensor_scalar(
                    diag_t[:, k, kb, :], ident[:], taps_t[:, kb, k:k + 1], None,
                    OP.mult)

        zb_t = pab.tile([128, NK, 2], F)
        nc.gpsimd.memset(zb_t[:], 0.0)
        onesb_t = pab.tile([128, 128], BF)
        onef = pab.tile([128, 128], F)
        nc.vector.memset(onef[:], 1.0)
        nc.vector.tensor_copy(onesb_t[:], onef[:])
        onecol_t = pab.tile([1, 128], FR)
        nc.vector.tensor_copy(onecol_t[:], onef[0:1, :])

        resT = pab.tile([R, S], F)         # resonance.T (unscaled)
        accT = pab.tile([R, S], F)         # scan output
        xmean = pab.tile([128, NK], F)
        nc.vector.memset(xmean[:], 0.0)
        p2row = pab.tile([1, S], F)        # 1/||h||^2 rows (staged)

        def newton_rsqrt(pool, ssq_ap, n, tag):
            """rsqrt(ssq/D + eps) via Newton from y0=1.5-0.5m (m near 1)."""
            m = pool.tile([128, n], F, tag=tag + "m")
            nc.vector.tensor_scalar(m[:], ssq_ap, 1.0 / D, EPS, OP.mult, OP.add)
            y = pool.tile([128, n], F, tag=tag + "y")
            nc.vector.tensor_scalar(y[:], m[:], -0.5, 1.5, OP.mult, OP.add)
            t1 = pool.tile([128, n], F, tag=tag + "t1")
            t2 = pool.tile([128, n], F, tag=tag + "t2")
            for _ in range(2):
                nc.vector.tensor_tensor(t1[:], y[:], y[:], OP.mult)
                nc.vector.tensor_tensor(t2[:], t1[:], m[:], OP.mult)
                nc.vector.tensor_scalar(t1[:], t2[:], -0.5, 1.5, OP.mult, OP.add)
                nc.vector.tensor_tensor(y[:], y[:], t1[:], OP.mult)
            return m, y

        # ================= PHASE A =================
        with ExitStack() as actx:
            pa = actx.enter_context(tc.tile_pool(name="pa", bufs=2))
            pa_sq = actx.enter_context(tc.tile_pool(name="pa_sq", bufs=2))
            pa_sc = actx.enter_context(tc.tile_pool(name="pa_sc", bufs=1))
            pa_xt = actx.enter_context(tc.tile_pool(name="pa_xt", bufs=5))
            pp_tr = actx.enter_context(
                tc.tile_pool(name="pp_tr", bufs=3, space="PSUM"))
            pp_conv = actx.enter_context(
                tc.tile_pool(name="pp_conv", bufs=2, space="PSUM"))
            pp_res = actx.enter_context(
                tc.tile_pool(name="pp_res", bufs=2, space="PSUM"))
            pp_l2 = actx.enter_context(
                tc.tile_pool(name="pp_l2", bufs=1, space="PSUM"))

            prev_xhT = None
            H = CH // 128
            for c in range(NCH):
                xhT = pa.tile([128, NK, CH + 2], FR, tag="xhT")
                if prev_xhT is None:
                    nc.vector.tensor_copy(xhT[:, :, 0:2], zb_t[:])
                else:
                    nc.vector.tensor_copy(xhT[:, :, 0:2],
                                          prev_xhT[:, :, CH:CH + 2])
                xts = []
                ssqt = pa.tile([128, H], F, tag="ssqt")
                for h in range(H):
                    s0 = c * CH + h * 128
                    xt = pa_xt.tile([128, D], F, tag="xt")
                    nc.sync.dma_start(xt[:], x_d.ap()[s0:s0 + 128, :])
                    sq_scr = pa_sc.tile([128, D], BF, tag="sqscr")
                    nc.scalar.activation(sq_scr[:], xt[:], AF.Square,
                                         accum_out=ssqt[:, h:h + 1])
                    xts.append(xt)
                _, rstdt = newton_rsqrt(pa, ssqt[:], H, "nA")
                for h in range(H):
                    xh = pa.tile([128, D], FR, tag="xh")
                    nc.vector.tensor_scalar(xh[:], xts[h][:],
                                            rstdt[:, h:h + 1], None, OP.mult)
                    for g in range(2):
                        trp = pp_tr.tile([128, 4, 128], FR, tag="trp")
                        for j in range(4):
                            kb = g * 4 + j
                            nc.tensor.transpose(
                                trp[:, j, :], xh[:, kb * 128:(kb + 1) * 128],
                                ident_r[:])
                        nc.vector.tensor_copy(
                            xhT[:, g * 4:(g + 1) * 4,
                                2 + h * 128:2 + h * 128 + 128],
                            trp[:])
                hT = pa.tile([128, NK, CH], RD, tag="hT")
                xm_st = pa.tile([128, NK], F, tag="xmst")
                for grp in range(4):
                    cps = pp_conv.tile([128, 2, CH], F, tag="cps")
                    for j in range(2):
                        kb = grp * 2 + j
                        for k in range(3):
                            nc.tensor.matmul(
                                cps[:, j, :], diag_t[:, k, kb, :],
                                xhT[:, kb, k:k + CH],
                                start=(k == 0), stop=(k == 2))
                    if c == 0:
                        nc.vector.tensor_tensor(
                            cps[:, :, 0:2], cps[:, :, 0:2],
                            corr_t[:, grp * 2:(grp + 1) * 2, :], OP.add)
                    for j in range(2):
                        kb = grp * 2 + j
                        nc.scalar.activation(
                            hT[:, kb, :], cps[:, j, :], AF.Silu,
                            bias=cbias_t[:, kb:kb + 1],
                            accum_out=xm_st[:, kb:kb + 1])
                nc.vector.tensor_tensor(xmean[:], xmean[:], xm_st[:], OP.add)
                sq2 = pa_sq.tile([128, NK, CH], BF, tag="sq2")
                nc.vector.tensor_tensor(sq2[:, 0:4, :], hT[:, 0:4, :],
                                        hT[:, 0:4, :], OP.mult)
                nc.gpsimd.tensor_tensor(sq2[:, 4:8, :], hT[:, 4:8, :],
                                        hT[:, 4:8, :], OP.mult)
                l2p = pp_l2.tile([128, CH], F, tag="l2p")
                for kb in range(NK):
                    nc.tensor.matmul(l2p[:], onesb_t[:], sq2[:, kb, :],
                                     start=(kb == 0), stop=(kb == NK - 1))
                nc.vector.reciprocal(p2row[0:1, c * CH:(c + 1) * CH],
                                     l2p[0:1, :])
                rps = pp_res.tile([R, CH], F, tag="rps")
                for kb in range(NK):
                    nc.tensor.matmul(rps[:], fnT_t[:, kb, :],
                                     hT[:, kb, :],
                                     start=(kb == 0), stop=(kb == NK - 1))
                nc.scalar.copy(resT[:, c * CH:(c + 1) * CH], rps[:])
                prev_xhT = xhT

        _PH = os.environ.get("PHASES", "abc")

        # ================= PHASE B =================
        with ExitStack() as bctx:
          if "b" in _PH:
            pb = bctx.enter_context(tc.tile_pool(name="pb", bufs=2))
            pb1 = bctx.enter_context(tc.tile_pool(name="pb1", bufs=1))
            pp_b = bctx.enter_context(
                tc.tile_pool(name="pp_b", bufs=2, space="PSUM"))
            pp_rl = bctx.enter_context(
                tc.tile_pool(name="pp_rl", bufs=2, space="PSUM"))
            for k in range(NK):
                nc.sync.dma_start(wgT_t[:, k, :],
                                  wgT_d.ap()[k * 128:(k + 1) * 128, :])
                nc.sync.dma_start(wrT_t[:, k, :],
                                  wrT_d.ap()[k * 128:(k + 1) * 128, :])
            nc.sync.dma_start(gbias_t[:], gbias_d.ap())
            nc.sync.dma_start(rbias_t[:], rbias_d.ap())
            nc.sync.dma_start(st0_t[:], st0_d.ap())
            rl2_fr = pb1.tile([1, S], FR)
            nc.scalar.activation(rl2_fr[:], p2row[:], AF.Sqrt)
            xm_s = pb1.tile([128, NK], F)
            nc.vector.tensor_scalar(xm_s[:], xmean[:], 1.0 / S, None, OP.mult)
            gps = pp_b.tile([R, 1], F)
            rps2 = pp_b.tile([R, 1], F)
            for k in range(NK):
                nc.tensor.matmul(gps[:], wgT_t[:, k, :], xm_s[:, k:k + 1],
                                 start=(k == 0), stop=(k == NK - 1))
            for k in range(NK):
                nc.tensor.matmul(rps2[:], wrT_t[:, k, :], xm_s[:, k:k + 1],
                                 start=(k == 0), stop=(k == NK - 1))
            gate_s = pb1.tile([R, 1], F)
            nc.scalar.activation(gate_s[:], gps[:], AF.Sigmoid,
                                 bias=gbias_t[:, 0:1])
            gmask = pb1.tile([R, 1], F)
            nc.vector.tensor_scalar(gmask[:], gate_s[:], 0.001, None, OP.is_ge)
            gate_t = pb1.tile([R, 1], F)
            nc.vector.tensor_tensor(gate_t[:], gate_s[:], gmask[:], OP.mult)
            ret_t = pb1.tile([R, 1], F)
            nc.scalar.activation(ret_t[:], rps2[:], AF.Sigmoid,
                                 bias=rbias_t[:, 0:1])
            ones_sc = pb1.tile([R, SC], F)
            nc.vector.memset(ones_sc[:], 1.0)
            ret_b = pb1.tile([R, SC], F)
            nc.vector.tensor_scalar(ret_b[:], ones_sc[:], ret_t[:, 0:1], None,
                                    OP.mult)
            for c in range(NSC):
                rl2b = pp_rl.tile([128, SC], F, tag="rl2b")
                for q in range(SC // 512):
                    nc.tensor.matmul(rl2b[:, q * 512:(q + 1) * 512],
                                     onecol_t[:],
                                     rl2_fr[:, c * SC + q * 512:
                                            c * SC + (q + 1) * 512],
                                     start=True, stop=True)
                sig = pb.tile([R, SC], F, tag="sig")
                nc.vector.scalar_tensor_tensor(
                    sig[:], resT[:, c * SC:(c + 1) * SC], gate_t[:, 0:1],
                    rl2b[:], OP.mult, OP.mult)
                init_ap = st0_t[:, 0:1] if c == 0 else accT[:, c * SC - 1:c * SC]
                nc.vector.tensor_tensor_scan(
                    accT[:, c * SC:(c + 1) * SC], ret_b[:], sig[:], init_ap,
                    OP.mult, OP.add)
                nc.scalar.copy(accTr[:, c * SC:(c + 1) * SC],
                               accT[:, c * SC:(c + 1) * SC])
            nc.sync.dma_start(stN_d.ap(), accT[:, S - 1:S])

        abctx.close()

        # ================= PHASE C =================
        with ExitStack() as cctx:
          if "c" in _PH:
            pc = cctx.enter_context(tc.tile_pool(name="pc", bufs=2))
            pc_sq = cctx.enter_context(tc.tile_pool(name="pc_sq", bufs=2))
            pp_op = cctx.enter_context(
                tc.tile_pool(name="pp_op", bufs=1, space="PSUM"))
            pp_tr2 = cctx.enter_context(
                tc.tile_pool(name="pp_tr2", bufs=2, space="PSUM"))
            pp_ffn = cctx.enter_context(
                tc.tile_pool(name="pp_ffn", bufs=2, space="PSUM"))
            pcw = cctx.enter_context(tc.tile_pool(name="pcw", bufs=1))

            wfT_t = pcw.tile([128, NK, D], FR)
            for k in range(NK):
                wf_f = pc.tile([128, D], F, tag="wfld")
                nc.sync.dma_start(wf_f[:], wfT_d.ap()[k * 128:(k + 1) * 128, :])
                nc.vector.tensor_copy(wfT_t[:, k, :], wf_f[:])
            fbias_t = pcw.tile([1, D], FR)
            fb_f = pc.tile([1, D], F, tag="fbld")
            nc.sync.dma_start(fb_f[:], fbias_d.ap())
            nc.vector.tensor_copy(fbias_t[:], fb_f[:])
            woT_t = pcw.tile([R, D], FR)
            wo_f = pc.tile([R, D], F, tag="wold")
            nc.sync.dma_start(wo_f[:], woT_d.ap())
            nc.vector.tensor_copy(woT_t[:], wo_f[:])
            for tp in range(NST // 2):
                x2s = []
                ssqp = pc.tile([128, 2], F, tag="ssqp")
                for half in range(2):
                    s0 = (tp * 2 + half) * ST
                    ops = pp_op.tile([128, D], F, tag="ops")
                    for n in range(2):
                        nc.tensor.matmul(ops[:, n * 512:(n + 1) * 512],
                                         accTr[:, s0:s0 + 128],
                                         woT_t[:, n * 512:(n + 1) * 512],
                                         start=True, stop=True)
                    xt2 = pc.tile([128, D], F, tag=f"xt2{half}")
                    nc.sync.dma_start(xt2[:], x_d.ap()[s0:s0 + 128, :])
                    x2 = pc.tile([128, D], F, tag=f"x2{half}")
                    nc.vector.tensor_tensor(x2[:], xt2[:], ops[:], OP.add)
                    sq_scr = pc_sq.tile([128, D], BF, tag="sqscr2")
                    nc.scalar.activation(sq_scr[:], x2[:], AF.Square,
                                         accum_out=ssqp[:, half:half + 1])
                    x2s.append(x2)
                mp, rstdp = newton_rsqrt(pc, ssqp[:], 2, "nC")
                sstp = pc.tile([128, 2], F, tag="sstp")
                nc.vector.reciprocal(sstp[:], rstdp[:])
                srows = []
                for half in range(2):
                    rowp = pp_tr2.tile([1, 128], F, tag="trp2")
                    nc.tensor.transpose(rowp[:], sstp[:, half:half + 1],
                                        ident[:])
                    sr = pc.tile([1, 128], FR, tag=f"srow{half}")
                    nc.vector.tensor_copy(sr[:], rowp[:])
                    srows.append(sr)
                for half in range(2):
                    x2 = x2s[half]
                    x2T = pc.tile([128, NK, 128], FR, tag="x2T")
                    for g in range(2):
                        trp = pp_tr2.tile([128, 4, 128], F, tag="trp2")
                        for j in range(4):
                            kb = g * 4 + j
                            nc.tensor.transpose(
                                trp[:, j, :], x2[:, kb * 128:(kb + 1) * 128],
                                ident[:])
                        nc.vector.tensor_copy(
                            x2T[:, g * 4:(g + 1) * 4, :], trp[:])
                    fps = pp_ffn.tile([128, D], F, tag="fps")
                    for n in range(2):
                        for k in range(NK):
                            nc.tensor.matmul(fps[:, n * 512:(n + 1) * 512],
                                             x2T[:, k, :],
                                             wfT_t[:, k, n * 512:(n + 1) * 512],
                                             start=(k == 0), stop=False)
                        nc.tensor.matmul(fps[:, n * 512:(n + 1) * 512],
                                         srows[half][:],
                                         fbias_t[:, n * 512:(n + 1) * 512],
                                         start=False, stop=True)
                    sil = pc.tile([128, D], F, tag="sil")
                    nc.scalar.activation(sil[:], fps[:], AF.Silu,
                                         scale=rstdp[:, half:half + 1])
                    yt = pc.tile([128, D], F, tag="yt")
                    nc.gpsimd.tensor_tensor(yt[:], x2[:], sil[:], OP.add)
                    s0 = (tp * 2 + half) * ST
                    nc.sync.dma_start(y_d.ap()[s0:s0 + 128, :], yt[:])

    nc.compile()
    return nc


def _prep(inputs):
    f32 = np.float32
    x = np.asarray(inputs["x"], f32)
    gamma = np.asarray(inputs["scale_gamma"], f32)
    beta = np.asarray(inputs["scale_beta"], f32)
    st0 = np.asarray(inputs["resonance_state"], f32)
    cw = np.asarray(inputs["conv_w"], f32)[:, 0, :]          # (D, 3)
    freq = np.asarray(inputs["frequencies"], f32)
    rbias = np.asarray(inputs["retention_bias"], f32)
    w_ret = np.asarray(inputs["w_ret"], f32)
    w_gate = np.asarray(inputs["w_gate"], f32)
    gbias = np.asarray(inputs["gate_bias"], f32)
    w_out = np.asarray(inputs["w_out"], f32)
    w_ffn = np.asarray(inputs["w_ffn"], f32)
    n1 = np.asarray(inputs["norm1_w"], f32)
    n2 = np.asarray(inputs["norm2_w"], f32)

    g1 = n1 * gamma
    taps = cw * g1[:, None]                                   # (D, 3)
    cbias = beta * cw.sum(1)                                  # (D,)
    corr0 = -beta * (cw[:, 0] + cw[:, 1])
    corr1 = -beta * cw[:, 0]

    def dlay(v):  # (D,) -> (128, NK)
        return np.ascontiguousarray(v.reshape(NK, 128).T)

    taps_a = np.ascontiguousarray(
        np.stack([dlay(taps[:, k]) for k in range(3)], axis=-1))  # (128,NK,3)
    corr_a = np.ascontiguousarray(
        np.stack([dlay(corr0), dlay(corr1)], axis=-1))            # (128,NK,2)

    fn = freq / np.maximum(np.linalg.norm(freq, axis=-1, keepdims=True), 1e-12)
    g2 = n2 * gamma
    wfT = np.ascontiguousarray(w_ffn.T * g2[:, None])
    fbias = (beta @ w_ffn.T).reshape(1, D)

    shared = dict(
        taps=taps_a, cbias=np.ascontiguousarray(dlay(cbias)), corr=corr_a,
        fnT=np.ascontiguousarray(fn.T), wgT=np.ascontiguousarray(w_gate.T),
        wrT=np.ascontiguousarray(w_ret.T),
        gbias=np.ascontiguousarray(gbias.reshape(R, 1)),
        rbias=np.ascontiguousarray(rbias.reshape(R, 1)),
        woT=np.ascontiguousarray(w_out.T), wfT=wfT,
        fbias=np.ascontiguousarray(fbias))
    in_maps = []
    for b in range(B):
        m = dict(shared)
        m["x"] = np.ascontiguousarray(x[b])
        m["state0"] = np.ascontiguousarray(st0[b].reshape(R, 1))
        in_maps.append(m)
    return in_maps


def kernel(**inputs):
    from concourse.bass_utils import run_bass_kernel_spmd
    if "nc" not in _CACHE:
        _CACHE["nc"] = _build()
    nc = _CACHE["nc"]
    in_maps = _prep(inputs)
    res = run_bass_kernel_spmd(nc, in_maps, core_ids=list(range(B)))
    y = np.stack([res.results[b]["y"] for b in range(B)])
    stN = np.stack([res.results[b]["stateN"][:, 0] for b in range(B)])
    return y, stN
